# revision 3
# baseline (speedup 1.0000x reference)
"""Cross-channel multi-head attention on 8 Trainium2 NeuronCores.

Sharding: data-parallel over the batch axis. bs2=16 sequences form bs=8
(batch, 2-channel) pairs; each core handles one pair fully locally
(cross-channel attention couples only the two channels of the same batch
element), so no collectives are needed.

Per core (T=2048 tokens = 2 channels x 1024 patches, D=1024, H=8 heads,
dk=128; heads 0..5 attend to the other channel's K/V, heads 6..7 to the
same channel):
  1. Phase A: V = x @ Wv + bv in natural [T, D] layout, stored per head
     with an extra ones column (softmax denominator trick).
  2. Phase B per head h: Qt/Kt = (x @ W)^T in [dk, T] layout; per
     channel: S^T[m,n] = Kt-slice^T x Qt-slice; P^T = exp(S^T/sqrt(dk));
     Z-slices = V-chunks contracted with P^T over m; denominators from
     the ones column, normalization fused into the PSUM->zn copy; PE
     transpose lands Zt in [dout, T] layout. The drain chain
     (reciprocal/normalize/transpose/copy) is software-pipelined two
     groups behind the matmuls so the PE never waits on the DVE.
  3. Phase C: out = Zt^T-slices @ Wo + bo, written bf16 (upcast on the
     host); Wo streams per-chunk into the k-outer loop so the first
     matmul only waits for one chunk.

All matmuls bf16 with f32 PSUM accumulation. The host pre-transposes x
to a [128, quarter, chunk, 512] layout so every DMA is per-partition
contiguous; all input DMAs ride one HWDGE queue in consumption order
(x/Wv chunks first, then Wq/Wk) so the phase-A stream is never starved
by weight traffic, with biases on the Act HWDGE ring in parallel.
"""

import sys

if "/opt/trn_rl_repo" not in sys.path:
    sys.path.insert(0, "/opt/trn_rl_repo")

import numpy as np
import ml_dtypes

import concourse.bass as bass
import concourse.tile as tile
from concourse import mybir
from concourse.bass_utils import run_bass_kernel_spmd
from concourse.masks import make_identity

# Walrus in this container rejects >1 wait condition on TPB_CTRL ops
# (Tile's kernel-tail drain carries one per active proc). Split them.
import os

_here = os.path.dirname(os.path.abspath(__file__))
if _here not in sys.path:
    sys.path.insert(0, _here)
try:
    import bir_legalize
except ImportError:  # graded in a bare dir: fall back to inline copy
    bir_legalize = None

N = 1024  # patches per channel
D = 1024
H = 8
DK = 128
N_CROSS = 6
T = 2 * N  # tokens per core (2 channels of one batch element)
P = 128
KO = D // P  # 8 outer chunks of the 1024-wide dims
TC = T // P  # 16 token chunks
NQ = 4  # quarters of the token axis (512 tokens each)
BF = mybir.dt.bfloat16
F32 = mybir.dt.float32
SCALE = 1.0 / float(np.sqrt(DK))

_CACHE = {}


def _legalize_install():
    if bir_legalize is not None:
        bir_legalize.install()
        return
    # Inline fallback (kernel.py must be self-contained when graded).
    import json
    import concourse.bass2jax as bass2jax
    from concourse.bass_utils import compile_bir_kernel as _orig

    if getattr(bass2jax.compile_bir_kernel, "_legalized", False):
        return

    OPCODE_MAX = {}
    SKIP = set()

    def _legalize(bir_json):
        d = json.loads(bir_json)
        changed = False
        for fn in d.get("functions", []):
            for bb in fn.get("blocks") or fn.get("basicblocks") or []:
                out = []
                for inst in bb.get("instructions", []):
                    sync = inst.get("sync_info") or {}
                    waits = sync.get("on_wait") or []
                    cap = OPCODE_MAX.get(inst.get("opcode"), 1)
                    if len(waits) > cap and inst.get("opcode") not in SKIP:
                        extra, keep = waits[:-cap], waits[-cap:]
                        for i, w in enumerate(extra):
                            out.append(
                                {
                                    "debug": inst.get("debug", 0),
                                    "engine": inst["engine"],
                                    "ins": [],
                                    "outs": [],
                                    "is_reset_sema": False,
                                    "name": f"{inst['name']}-sw{i}",
                                    "opcode": "Drain",
                                    "sync_info": {"on_update": [], "on_wait": [w]},
                                }
                            )
                        sync["on_wait"] = keep
                        inst["sync_info"] = sync
                        changed = True
                    out.append(inst)
                bb["instructions"] = out
        return json.dumps(d).encode() if changed else bir_json

    def compile_bir_kernel(bir_json, tmpdir, neff_name="file.neff"):
        return _orig(_legalize(bir_json), tmpdir, neff_name)

    compile_bir_kernel._legalized = True
    bass2jax.compile_bir_kernel = compile_bir_kernel


def _build():
    nc = bass.Bass()

    xt_d = nc.dram_tensor("xt", [P, NQ, KO, 512], BF, kind="ExternalInput").ap()
    wq_d = nc.dram_tensor("wq", [P, KO, D], BF, kind="ExternalInput").ap()
    wk_d = nc.dram_tensor("wk", [P, KO, D], BF, kind="ExternalInput").ap()
    wv_d = nc.dram_tensor("wv", [P, KO, D], BF, kind="ExternalInput").ap()
    wo_d = nc.dram_tensor("wo", [P, KO, D], BF, kind="ExternalInput").ap()
    bqk_d = nc.dram_tensor("bqk", [P, 2 * KO], F32, kind="ExternalInput").ap()
    bvr_d = nc.dram_tensor("bvr", [P, D], F32, kind="ExternalInput").ap()
    bor_d = nc.dram_tensor("bor", [P, D], F32, kind="ExternalInput").ap()
    out_d = nc.dram_tensor("out", [T, D], BF, kind="ExternalOutput").ap()

    with tile.TileContext(nc) as tc:
        with (
            tc.tile_pool(name="consts", bufs=1) as consts,
            tc.tile_pool(name="big", bufs=1) as big,
        ):
            ident = consts.tile([P, P], BF)
            make_identity(nc, ident)
            bqk_sb = consts.tile([P, 2 * KO], F32)
            warm_in = consts.tile([P, P], BF)
            nc.vector.memset(warm_in[:], 0.0)
            warm_rhs = consts.tile([P, 512], BF)
            nc.vector.memset(warm_rhs[:], 0.0)

            Vg = big.tile([P, TC, H, DK + 1], BF)  # natural V + ones col
            nc.vector.memset(Vg[:, :, :, DK : DK + 1], 1.0)
            Zt = big.tile([P, KO, T], BF)  # attention out, [dout, T]

            with tc.tile_pool(name="xt_w", bufs=1) as xt_w:
                Xt = xt_w.tile([P, NQ, KO, 512], BF)
                Wq = xt_w.tile([P, KO, D], BF)
                Wk = xt_w.tile([P, KO, D], BF)

                # ---- phase A: V projection ----
                wv_ctx = tc.tile_pool(name="wv_pool", bufs=1)
                wv_pool = wv_ctx.__enter__()
                Wv = wv_pool.tile([P, KO, D], BF)
                bv_r = wv_pool.tile([P, D], F32)

                # Biases on the Act HWDGE ring (parallel to the main
                # input stream on the SP ring).
                nc.scalar.dma_start(bv_r[:], bvr_d)
                nc.scalar.dma_start(bqk_sb[:], bqk_d)

                # Main input stream, in consumption order: quarter-0 x
                # chunks paired with Wv chunks (phase A base 0), the
                # remaining x quarters, then Wq/Wk for phase B.
                nc.sync.dma_start(Wv[:, 0, :], wv_d[:, 0, :])
                nc.sync.dma_start(Xt[:, 0, 0, :], xt_d[:, 0, 0, :])
                for k in range(1, KO):
                    nc.sync.dma_start(Wv[:, k, :], wv_d[:, k, :])
                    nc.sync.dma_start(Xt[:, 0, k, :], xt_d[:, 0, k, :])
                for q in range(1, NQ):
                    nc.sync.dma_start(Xt[:, q, :, :], xt_d[:, q, :, :])
                for k in range(KO):
                    nc.sync.dma_start(Wq[:, k, :], wq_d[:, k, :])
                    nc.sync.dma_start(Wk[:, k, :], wk_d[:, k, :])

                with tc.tile_pool(name="psA", bufs=8, space="PSUM") as psA:
                    # Warm the PE HAM clock gate with throwaway
                    # accumulating matmul groups so the first real
                    # matmuls run at full clock while DMA chunks land.
                    for g in range(2):
                        wps = psA.tile([P, 512], F32, tag="a")
                        for k in range(12):
                            nc.tensor.matmul(
                                wps[:],
                                warm_in[:],
                                warm_rhs[:],
                                start=(k == 0),
                                stop=(k == 11),
                            )

                    # 4 bases x 8 concurrent PSUM groups (all 8 banks),
                    # k-outer so each (x, Wv) chunk pair is consumed
                    # across the whole base as soon as it lands.
                    groups = [(tci, dh) for tci in range(TC) for dh in range(2)]
                    for base in range(0, len(groups), 8):
                        tiles = [
                            psA.tile([P, 512], F32, tag="a", name=f"vps_{base}_{g}")
                            for g in range(8)
                        ]
                        for k in range(KO):
                            for g in range(8):
                                tci, dh = groups[base + g]
                                nc.tensor.matmul(
                                    tiles[g][:],
                                    Xt[
                                        :,
                                        tci // 4,
                                        k,
                                        (tci % 4) * P : (tci % 4 + 1) * P,
                                    ],
                                    Wv[:, k, dh * 512 : (dh + 1) * 512],
                                    start=(k == 0),
                                    stop=(k == KO - 1),
                                )
                        for g in range(8):
                            tci, dh = groups[base + g]
                            nc.vector.tensor_tensor(
                                Vg[:, tci, 4 * dh : 4 * dh + 4, :DK],
                                tiles[g].rearrange("p (h d) -> p h d", d=DK),
                                bv_r[:, dh * 512 : (dh + 1) * 512].rearrange(
                                    "p (h d) -> p h d", d=DK
                                ),
                                mybir.AluOpType.add,
                            )

                wv_ctx.__exit__(None, None, None)

                # ---- phase B: per-head Q/K projection + attention ----
                with (
                    tc.tile_pool(name="qk", bufs=1) as qk,
                    tc.tile_pool(name="pt_pool", bufs=2) as pt_pool,
                    tc.tile_pool(name="att_sm", bufs=4) as att_sm,
                    tc.tile_pool(name="ps_a", bufs=4, space="PSUM") as ps_a,
                    tc.tile_pool(name="ps_z", bufs=2, space="PSUM") as ps_z,
                    tc.tile_pool(name="ps_zt", bufs=2, space="PSUM") as ps_zt,
                ):

                    def proj_head(h, w_sb, bias_col):
                        dst = qk.tile([P, T], BF, tag="qth" if w_sb is Wq else "kth")
                        for tt in range(NQ):
                            ps = ps_a.tile([P, 512], F32, tag="ps")
                            for k in range(KO):
                                nc.tensor.matmul(
                                    ps[:],
                                    w_sb[:, k, h * P : (h + 1) * P],
                                    Xt[:, tt, k, :],
                                    start=(k == 0),
                                    stop=(k == KO - 1),
                                )
                            nc.vector.tensor_tensor(
                                dst[:, tt * 512 : (tt + 1) * 512],
                                ps[:],
                                bqk_sb[:, bias_col : bias_col + 1].to_broadcast(
                                    (P, 512)
                                ),
                                mybir.AluOpType.add,
                            )
                        return dst

                    def scores_unit(h, ch, Qth, Kth):
                        chp = (1 - ch) if h < N_CROSS else ch  # kv channel
                        q0 = ch * N
                        m0 = chp * N
                        PT = pt_pool.tile([P, KO, N], BF, tag="pt")
                        for mi in range(KO):
                            for nh in range(2):
                                ps = ps_a.tile([P, 512], F32, tag="ps")
                                nc.tensor.matmul(
                                    ps[:],
                                    Kth[:, m0 + mi * P : m0 + (mi + 1) * P],
                                    Qth[:, q0 + nh * 512 : q0 + (nh + 1) * 512],
                                    start=True,
                                    stop=True,
                                )
                                nc.scalar.activation(
                                    PT[:, mi, nh * 512 : (nh + 1) * 512],
                                    ps[:],
                                    mybir.ActivationFunctionType.Exp,
                                    scale=SCALE,
                                )
                        return (PT, h, ch, chp, q0)

                    def attnv_unit(state):
                        PT, h, ch, chp, q0 = state

                        def drain(psz):
                            r = att_sm.tile([P, 1], F32, tag="r")
                            nc.vector.reciprocal(r[:], psz[:, DK : DK + 1])
                            zn = att_sm.tile([P, DK], BF, tag="zn")
                            nc.vector.tensor_tensor(
                                zn[:],
                                psz[:, :DK],
                                r[:, 0:1].to_broadcast((P, DK)),
                                mybir.AluOpType.mult,
                            )
                            pzt = ps_zt.tile([P, P], BF, tag="zt")
                            nc.tensor.transpose(pzt[:], zn[:], ident[:])
                            return pzt

                        def store(pzt, ni):
                            nc.vector.tensor_copy(
                                Zt[:, h, q0 + ni * P : q0 + (ni + 1) * P], pzt[:]
                            )

                        pend = []  # [(psz|pzt, ni), ...] pipelined drains
                        for ni in range(KO):
                            psz = ps_z.tile([P, DK + 1], F32, tag="z")
                            for mi in range(KO):
                                nc.tensor.matmul(
                                    psz[:],
                                    PT[:, mi, ni * P : (ni + 1) * P],
                                    Vg[:, chp * KO + mi, h, :],
                                    start=(mi == 0),
                                    stop=(mi == KO - 1),
                                )
                            if ni >= 1:
                                pend[ni - 1] = (drain(pend[ni - 1][0]), ni - 1)
                            if ni >= 2:
                                store(*pend[ni - 2])
                            pend.append((psz, ni))
                        pend[KO - 1] = (drain(pend[KO - 1][0]), KO - 1)
                        store(*pend[KO - 2])
                        store(*pend[KO - 1])

                    prev = None
                    for h in range(H):
                        Qth = proj_head(h, Wq, h)
                        Kth = proj_head(h, Wk, KO + h)
                        for ch in range(2):
                            cur = scores_unit(h, ch, Qth, Kth)
                            if prev is not None:
                                attnv_unit(prev)
                            prev = cur
                    attnv_unit(prev)

            # ---- phase C: output projection ----
            with (
                tc.tile_pool(name="wo_pool", bufs=1) as wo_pool,
                tc.tile_pool(name="y_pool", bufs=6) as y_pool,
                tc.tile_pool(name="ps_y", bufs=8, space="PSUM") as ps_y,
            ):
                Wo = wo_pool.tile([P, KO, D], BF)
                bo_r = wo_pool.tile([P, D], F32)
                nc.scalar.dma_start(bo_r[:], bor_d)
                for k in range(KO):
                    nc.sync.dma_start(Wo[:, k, :], wo_d[:, k, :])

                groups = [(tci, dh) for tci in range(TC) for dh in range(2)]
                for base in range(0, len(groups), 8):
                    tiles = [
                        ps_y.tile([P, 512], F32, tag="y", name=f"yps_{base}_{g}")
                        for g in range(8)
                    ]
                    for k in range(KO):
                        for g in range(8):
                            tci, dh = groups[base + g]
                            nc.tensor.matmul(
                                tiles[g][:],
                                Zt[:, k, tci * P : (tci + 1) * P],
                                Wo[:, k, dh * 512 : (dh + 1) * 512],
                                start=(k == 0),
                                stop=(k == KO - 1),
                            )
                    ycur = None
                    for g in range(8):
                        tci, dh = groups[base + g]
                        if dh == 0:
                            ycur = y_pool.tile([P, D], BF, tag="yt")
                        nc.vector.tensor_tensor(
                            ycur[:, dh * 512 : (dh + 1) * 512],
                            tiles[g][:],
                            bo_r[:, dh * 512 : (dh + 1) * 512],
                            mybir.AluOpType.add,
                        )
                        if dh == 1:
                            nc.sync.dma_start(
                                out_d[tci * P : (tci + 1) * P, :], ycur[:]
                            )
    return nc


def _get_program():
    if "nc" not in _CACHE:
        _legalize_install()
        _CACHE["nc"] = _build()
    return _CACHE["nc"]


def make_in_maps(inputs):
    x = np.asarray(inputs["x"], dtype=np.float32)
    bs2 = x.shape[0]
    n_cores = bs2 // 2
    bf = ml_dtypes.bfloat16

    weights = {}
    for name in ("Wq", "Wk", "Wv", "Wo"):
        w = np.asarray(inputs[name], dtype=np.float32)
        weights[name] = np.ascontiguousarray(
            w.reshape(KO, P, D).transpose(1, 0, 2)
        ).astype(bf)
    b = {k: np.asarray(inputs[k], dtype=np.float32) for k in ("bq", "bk", "bv", "bo")}
    bqk = np.ascontiguousarray(
        np.concatenate([b["bq"].reshape(KO, P).T, b["bk"].reshape(KO, P).T], axis=1)
    )
    bvr = np.ascontiguousarray(np.broadcast_to(b["bv"], (P, D)))
    bor = np.ascontiguousarray(np.broadcast_to(b["bo"], (P, D)))

    in_maps = []
    for c in range(n_cores):
        xT = x[2 * c : 2 * c + 2].reshape(T, D).T  # [D, T]
        xt = np.ascontiguousarray(
            xT.reshape(KO, P, NQ, 512).transpose(1, 2, 0, 3)
        ).astype(bf)
        in_maps.append(
            {
                "xt": xt,
                "wq": weights["Wq"],
                "wk": weights["Wk"],
                "wv": weights["Wv"],
                "wo": weights["Wo"],
                "bqk": bqk,
                "bvr": bvr,
                "bor": bor,
            }
        )
    return in_maps


def kernel(**inputs):
    bs2 = np.asarray(inputs["x"]).shape[0]
    n_cores = bs2 // 2
    in_maps = make_in_maps(inputs)
    nc = _get_program()
    res = run_bass_kernel_spmd(nc, in_maps, core_ids=list(range(n_cores)))
    out = np.empty((bs2, N, D), dtype=np.float32)
    for c in range(n_cores):
        out[2 * c : 2 * c + 2] = (
            res.results[c]["out"].astype(np.float32).reshape(2, N, D)
        )
    return out


# revision 4
# speedup vs baseline: 1.0585x; 1.0585x over previous
"""Cross-channel multi-head attention on 8 Trainium2 NeuronCores.

Sharding: data-parallel over the batch axis. bs2=16 sequences form bs=8
(batch, 2-channel) pairs; each core handles one pair fully locally
(cross-channel attention couples only the two channels of the same batch
element), so no collectives are needed.

Per core (T=2048 tokens = 2 channels x 1024 patches, D=1024, H=8 heads,
dk=128; heads 0..5 attend to the other channel's K/V, heads 6..7 to the
same channel):
  1. Phase A: V = x @ Wv + bv in natural [T, D] layout, stored per head
     with an extra ones column (softmax denominator trick).
  2. Phase B: per head h the work is round-robined at stretch level --
     scores(h,0) / attnv(h-1,1) / proj(h+1,Q) / scores(h,1) / attnv(h,0)
     / proj(h+1,K) -- so the Act engine's exp stream (the slowest
     per-unit engine) always has 2x PE slack and never blocks a matmul.
     Scores: S^T[m,n] = Kt-slice^T x Qt-slice; P^T = exp(S^T/sqrt(dk));
     attnv: Z-chunks = V-chunks contracted with P^T over m; denominators
     from the ones column; the drain chain (reciprocal/normalize/PE
     transpose into Zt) is software-pipelined two groups behind the
     matmuls.
  3. Phase C: out = Zt^T-slices @ Wo + bo, written bf16 on the Act HWDGE
     ring (host upcasts); Wo chunks 0-3 are preloaded at startup, 4-7
     stream in at C start ahead of the k-outer loop.

All matmuls bf16 with f32 PSUM accumulation. The host pre-arranges x and
the weights so every DMA is >=4KB-per-partition contiguous; all input
DMAs ride one HWDGE queue in consumption order (x/Wv first, then Wq/Wk,
then Wo-low), with biases and output on the Act HWDGE ring.
"""

import sys

if "/opt/trn_rl_repo" not in sys.path:
    sys.path.insert(0, "/opt/trn_rl_repo")

import numpy as np
import ml_dtypes

import concourse.bass as bass
import concourse.tile as tile
from concourse import mybir
from concourse.bass_utils import run_bass_kernel_spmd
from concourse.masks import make_identity

# Walrus in this container rejects >1 wait condition on TPB_CTRL ops
# (Tile's kernel-tail drain carries one per active proc). Split them.
import os

_here = os.path.dirname(os.path.abspath(__file__))
if _here not in sys.path:
    sys.path.insert(0, _here)
try:
    import bir_legalize
except ImportError:  # graded in a bare dir: fall back to inline copy
    bir_legalize = None

N = 1024  # patches per channel
D = 1024
H = 8
DK = 128
N_CROSS = 6
T = 2 * N  # tokens per core (2 channels of one batch element)
P = 128
KO = D // P  # 8 outer chunks of the 1024-wide dims
TC = T // P  # 16 token chunks
NQ = 4  # quarters of the token axis (512 tokens each)
BF = mybir.dt.bfloat16
F32 = mybir.dt.float32
SCALE = 1.0 / float(np.sqrt(DK))

_CACHE = {}


def _legalize_install():
    if bir_legalize is not None:
        bir_legalize.install()
        return
    # Inline fallback (kernel.py must be self-contained when graded).
    import json
    import concourse.bass2jax as bass2jax
    from concourse.bass_utils import compile_bir_kernel as _orig

    if getattr(bass2jax.compile_bir_kernel, "_legalized", False):
        return

    OPCODE_MAX = {}
    SKIP = set()

    def _legalize(bir_json):
        d = json.loads(bir_json)
        changed = False
        for fn in d.get("functions", []):
            for bb in fn.get("blocks") or fn.get("basicblocks") or []:
                out = []
                for inst in bb.get("instructions", []):
                    sync = inst.get("sync_info") or {}
                    waits = sync.get("on_wait") or []
                    cap = OPCODE_MAX.get(inst.get("opcode"), 1)
                    if len(waits) > cap and inst.get("opcode") not in SKIP:
                        extra, keep = waits[:-cap], waits[-cap:]
                        for i, w in enumerate(extra):
                            out.append(
                                {
                                    "debug": inst.get("debug", 0),
                                    "engine": inst["engine"],
                                    "ins": [],
                                    "outs": [],
                                    "is_reset_sema": False,
                                    "name": f"{inst['name']}-sw{i}",
                                    "opcode": "Drain",
                                    "sync_info": {"on_update": [], "on_wait": [w]},
                                }
                            )
                        sync["on_wait"] = keep
                        inst["sync_info"] = sync
                        changed = True
                    out.append(inst)
                bb["instructions"] = out
        return json.dumps(d).encode() if changed else bir_json

    def compile_bir_kernel(bir_json, tmpdir, neff_name="file.neff"):
        return _orig(_legalize(bir_json), tmpdir, neff_name)

    compile_bir_kernel._legalized = True
    bass2jax.compile_bir_kernel = compile_bir_kernel


def _build():
    nc = bass.Bass()

    xt_d = nc.dram_tensor("xt", [P, NQ, KO, 512], BF, kind="ExternalInput").ap()
    wq_d = nc.dram_tensor("wq", [P, KO, D], BF, kind="ExternalInput").ap()
    wk_d = nc.dram_tensor("wk", [P, KO, D], BF, kind="ExternalInput").ap()
    wv_d = nc.dram_tensor("wv", [P, KO, D], BF, kind="ExternalInput").ap()
    wo_d = nc.dram_tensor("wo", [P, KO, D], BF, kind="ExternalInput").ap()
    bqk_d = nc.dram_tensor("bqk", [P, 2 * KO], F32, kind="ExternalInput").ap()
    bvr_d = nc.dram_tensor("bvr", [P, D], F32, kind="ExternalInput").ap()
    bor_d = nc.dram_tensor("bor", [P, D], F32, kind="ExternalInput").ap()
    out_d = nc.dram_tensor("out", [T, D], BF, kind="ExternalOutput").ap()

    with tile.TileContext(nc) as tc:
        with (
            tc.tile_pool(name="consts", bufs=1) as consts,
            tc.tile_pool(name="big", bufs=1) as big,
        ):
            ident = consts.tile([P, P], BF)
            make_identity(nc, ident)
            bqk_sb = consts.tile([P, 2 * KO], F32)
            warm_in = consts.tile([P, P], BF)
            nc.vector.memset(warm_in[:], 0.0)
            warm_rhs = consts.tile([P, 512], BF)
            nc.vector.memset(warm_rhs[:], 0.0)

            Vg = big.tile([P, TC, H, DK + 1], BF)  # natural V + ones col
            nc.vector.memset(Vg[:, :, :, DK : DK + 1], 1.0)
            Zt = big.tile([P, KO, T], BF)  # attention out, [dout, T]
            Wo_lo = big.tile([P, 4, D], BF)  # Wo chunks 0-3, preloaded

            with (
                tc.tile_pool(name="xt_w", bufs=1) as xt_w,
                tc.tile_pool(name="qk2", bufs=2) as qk2,
                tc.tile_pool(name="qk1", bufs=1) as qk1,
                tc.tile_pool(name="pt_pool", bufs=2) as pt_pool,
                tc.tile_pool(name="att_sm", bufs=4) as att_sm,
                tc.tile_pool(name="ps1", bufs=2, space="PSUM") as ps1,
                tc.tile_pool(name="ps_s", bufs=2, space="PSUM") as ps_s,
                tc.tile_pool(name="ps_z", bufs=2, space="PSUM") as ps_z,
                tc.tile_pool(name="ps_zt", bufs=2, space="PSUM") as ps_zt,
            ):
                Xt = xt_w.tile([P, NQ, KO, 512], BF)
                Wq = xt_w.tile([P, KO, D], BF)
                Wk = xt_w.tile([P, KO, D], BF)

                # ---- phase A: V projection ----
                wv_ctx = tc.tile_pool(name="wv_pool", bufs=1)
                wv_pool = wv_ctx.__enter__()
                Wv = wv_pool.tile([P, KO, D], BF)
                bv_r = wv_pool.tile([P, D], F32)

                # Biases on the Act HWDGE ring (parallel to the main
                # input stream on the SP ring).
                nc.scalar.dma_start(bv_r[:], bvr_d)
                nc.scalar.dma_start(bqk_sb[:], bqk_d)

                # Main input stream, in consumption order, >=4KB rows.
                nc.sync.dma_start(Wv[:, 0:2, :], wv_d[:, 0:2, :])
                nc.sync.dma_start(Xt[:, 0, 0:4, :], xt_d[:, 0, 0:4, :])
                nc.sync.dma_start(Wv[:, 2:4, :], wv_d[:, 2:4, :])
                nc.sync.dma_start(Xt[:, 0, 4:8, :], xt_d[:, 0, 4:8, :])
                nc.sync.dma_start(Wv[:, 4:6, :], wv_d[:, 4:6, :])
                nc.sync.dma_start(Wv[:, 6:8, :], wv_d[:, 6:8, :])
                for q in range(1, NQ):
                    nc.sync.dma_start(Xt[:, q, :, :], xt_d[:, q, :, :])
                for j in range(0, KO, 2):
                    nc.sync.dma_start(Wq[:, j : j + 2, :], wq_d[:, j : j + 2, :])
                    nc.sync.dma_start(Wk[:, j : j + 2, :], wk_d[:, j : j + 2, :])
                nc.sync.dma_start(Wo_lo[:], wo_d[:, 0:4, :])

                # Warm the PE HAM clock gate with throwaway accumulating
                # matmul groups so the first real matmuls run at full
                # clock while the first DMA chunks land.
                for g in range(2):
                    wps = ps_s.tile([P, 512], F32, tag="s")
                    for k in range(12):
                        nc.tensor.matmul(
                            wps[:],
                            warm_in[:],
                            warm_rhs[:],
                            start=(k == 0),
                            stop=(k == 11),
                        )

                # 4 bases x 8 concurrent PSUM groups (borrowing every
                # pool's banks), k-outer so each (x, Wv) chunk pair is
                # consumed across the whole base as soon as it lands.
                groups = [(tci, dh) for tci in range(TC) for dh in range(2)]
                gpools = [ps1, ps1, ps_s, ps_s, ps_z, ps_z, ps_zt, ps_zt]
                gtags = ["ps1", "ps1", "s", "s", "z", "z", "zt", "zt"]
                for base in range(0, len(groups), 8):
                    tiles = [
                        gpools[g].tile(
                            [P, 512], F32, tag=gtags[g], name=f"vps_{base}_{g}"
                        )
                        for g in range(8)
                    ]
                    for k in range(KO):
                        for g in range(8):
                            tci, dh = groups[base + g]
                            nc.tensor.matmul(
                                tiles[g][:],
                                Xt[:, tci // 4, k, (tci % 4) * P : (tci % 4 + 1) * P],
                                Wv[:, k, dh * 512 : (dh + 1) * 512],
                                start=(k == 0),
                                stop=(k == KO - 1),
                            )
                    for g in range(8):
                        tci, dh = groups[base + g]
                        nc.vector.tensor_tensor(
                            Vg[:, tci, 4 * dh : 4 * dh + 4, :DK],
                            tiles[g].rearrange("p (h d) -> p h d", d=DK),
                            bv_r[:, dh * 512 : (dh + 1) * 512].rearrange(
                                "p (h d) -> p h d", d=DK
                            ),
                            mybir.AluOpType.add,
                        )

                wv_ctx.__exit__(None, None, None)

                # ---- phase B ----
                def proj_head(h, w_sb, bias_col):
                    dst = (qk2 if w_sb is Wq else qk1).tile(
                        [P, T], BF, tag="qth" if w_sb is Wq else "kth"
                    )
                    for tt in range(NQ):
                        ps = ps1.tile([P, 512], F32, tag="ps1")
                        for k in range(KO):
                            nc.tensor.matmul(
                                ps[:],
                                w_sb[:, k, h * P : (h + 1) * P],
                                Xt[:, tt, k, :],
                                start=(k == 0),
                                stop=(k == KO - 1),
                            )
                        nc.vector.tensor_tensor(
                            dst[:, tt * 512 : (tt + 1) * 512],
                            ps[:],
                            bqk_sb[:, bias_col : bias_col + 1].to_broadcast((P, 512)),
                            mybir.AluOpType.add,
                        )
                    return dst

                def scores_unit(h, ch, Qth, Kth):
                    chp = (1 - ch) if h < N_CROSS else ch  # kv channel
                    q0 = ch * N
                    m0 = chp * N
                    PT = pt_pool.tile([P, KO, N], BF, tag="pt")
                    for mi in range(KO):
                        for nh in range(2):
                            ps = ps_s.tile([P, 512], F32, tag="s")
                            nc.tensor.matmul(
                                ps[:],
                                Kth[:, m0 + mi * P : m0 + (mi + 1) * P],
                                Qth[:, q0 + nh * 512 : q0 + (nh + 1) * 512],
                                start=True,
                                stop=True,
                            )
                            nc.scalar.activation(
                                PT[:, mi, nh * 512 : (nh + 1) * 512],
                                ps[:],
                                mybir.ActivationFunctionType.Exp,
                                scale=SCALE,
                            )
                    return (PT, h, ch, chp, q0)

                def attnv_unit(state):
                    PT, h, ch, chp, q0 = state

                    def drain(psz):
                        r = att_sm.tile([P, 1], F32, tag="r")
                        nc.vector.reciprocal(r[:], psz[:, DK : DK + 1])
                        zn = att_sm.tile([P, DK], BF, tag="zn")
                        nc.vector.tensor_tensor(
                            zn[:],
                            psz[:, :DK],
                            r[:, 0:1].to_broadcast((P, DK)),
                            mybir.AluOpType.mult,
                        )
                        pzt = ps_zt.tile([P, P], BF, tag="zt")
                        nc.tensor.transpose(pzt[:], zn[:], ident[:])
                        return pzt

                    def store(pzt, ni):
                        nc.vector.tensor_copy(
                            Zt[:, h, q0 + ni * P : q0 + (ni + 1) * P], pzt[:]
                        )

                    pend = []
                    for ni in range(KO):
                        psz = ps_z.tile([P, DK + 1], F32, tag="z")
                        for mi in range(KO):
                            nc.tensor.matmul(
                                psz[:],
                                PT[:, mi, ni * P : (ni + 1) * P],
                                Vg[:, chp * KO + mi, h, :],
                                start=(mi == 0),
                                stop=(mi == KO - 1),
                            )
                        if ni >= 1:
                            pend[ni - 1] = (drain(pend[ni - 1][0]), ni - 1)
                        if ni >= 2:
                            store(*pend[ni - 2])
                        pend.append((psz, ni))
                    pend[KO - 1] = (drain(pend[KO - 1][0]), KO - 1)
                    store(*pend[KO - 2])
                    store(*pend[KO - 1])

                # Stretch-level round-robin: the Act engine's exps for
                # scores(h,ch) drain during the two following Act-free
                # stretches (attnv + proj), so matmuls never wait.
                Qth = proj_head(0, Wq, 0)
                Kth = proj_head(0, Wk, KO + 0)
                prev1 = None  # state (h-1, 1)
                for h in range(H):
                    cur0 = scores_unit(h, 0, Qth, Kth)
                    if prev1 is not None:
                        attnv_unit(prev1)
                    Qn = proj_head(h + 1, Wq, h + 1) if h + 1 < H else None
                    cur1 = scores_unit(h, 1, Qth, Kth)
                    attnv_unit(cur0)
                    Kn = proj_head(h + 1, Wk, KO + h + 1) if h + 1 < H else None
                    Qth, Kth = Qn, Kn
                    prev1 = cur1
                attnv_unit(prev1)

            # ---- phase C: output projection ----
            with (
                tc.tile_pool(name="wo_pool", bufs=1) as wo_pool,
                tc.tile_pool(name="y_pool", bufs=6) as y_pool,
                tc.tile_pool(name="ps_y", bufs=8, space="PSUM") as ps_y,
            ):
                Wo_hi = wo_pool.tile([P, 4, D], BF)
                bo_r = wo_pool.tile([P, D], F32)
                nc.scalar.dma_start(bo_r[:], bor_d)
                nc.sync.dma_start(Wo_hi[:, 0:2, :], wo_d[:, 4:6, :])
                nc.sync.dma_start(Wo_hi[:, 2:4, :], wo_d[:, 6:8, :])

                groups = [(tci, dh) for tci in range(TC) for dh in range(2)]
                for base in range(0, len(groups), 8):
                    tiles = [
                        ps_y.tile([P, 512], F32, tag="y", name=f"yps_{base}_{g}")
                        for g in range(8)
                    ]
                    for k in range(KO):
                        w_sb = Wo_lo if k < 4 else Wo_hi
                        for g in range(8):
                            tci, dh = groups[base + g]
                            nc.tensor.matmul(
                                tiles[g][:],
                                Zt[:, k, tci * P : (tci + 1) * P],
                                w_sb[:, k % 4, dh * 512 : (dh + 1) * 512],
                                start=(k == 0),
                                stop=(k == KO - 1),
                            )
                    ycur = None
                    for g in range(8):
                        tci, dh = groups[base + g]
                        if dh == 0:
                            ycur = y_pool.tile([P, D], BF, tag="yt")
                        nc.vector.tensor_tensor(
                            ycur[:, dh * 512 : (dh + 1) * 512],
                            tiles[g][:],
                            bo_r[:, dh * 512 : (dh + 1) * 512],
                            mybir.AluOpType.add,
                        )
                        if dh == 1:
                            nc.scalar.dma_start(
                                out_d[tci * P : (tci + 1) * P, :], ycur[:]
                            )
    return nc


def _get_program():
    if "nc" not in _CACHE:
        _legalize_install()
        _CACHE["nc"] = _build()
    return _CACHE["nc"]


def make_in_maps(inputs):
    x = np.asarray(inputs["x"], dtype=np.float32)
    bs2 = x.shape[0]
    n_cores = bs2 // 2
    bf = ml_dtypes.bfloat16

    weights = {}
    for name in ("Wq", "Wk", "Wv", "Wo"):
        w = np.asarray(inputs[name], dtype=np.float32)
        weights[name] = np.ascontiguousarray(
            w.reshape(KO, P, D).transpose(1, 0, 2)
        ).astype(bf)
    b = {k: np.asarray(inputs[k], dtype=np.float32) for k in ("bq", "bk", "bv", "bo")}
    bqk = np.ascontiguousarray(
        np.concatenate([b["bq"].reshape(KO, P).T, b["bk"].reshape(KO, P).T], axis=1)
    )
    bvr = np.ascontiguousarray(np.broadcast_to(b["bv"], (P, D)))
    bor = np.ascontiguousarray(np.broadcast_to(b["bo"], (P, D)))

    in_maps = []
    for c in range(n_cores):
        xT = x[2 * c : 2 * c + 2].reshape(T, D).T  # [D, T]
        xt = np.ascontiguousarray(
            xT.reshape(KO, P, NQ, 512).transpose(1, 2, 0, 3)
        ).astype(bf)
        in_maps.append(
            {
                "xt": xt,
                "wq": weights["Wq"],
                "wk": weights["Wk"],
                "wv": weights["Wv"],
                "wo": weights["Wo"],
                "bqk": bqk,
                "bvr": bvr,
                "bor": bor,
            }
        )
    return in_maps


def kernel(**inputs):
    bs2 = np.asarray(inputs["x"]).shape[0]
    n_cores = bs2 // 2
    in_maps = make_in_maps(inputs)
    nc = _get_program()
    res = run_bass_kernel_spmd(nc, in_maps, core_ids=list(range(n_cores)))
    out = np.empty((bs2, N, D), dtype=np.float32)
    for c in range(n_cores):
        out[2 * c : 2 * c + 2] = (
            res.results[c]["out"].astype(np.float32).reshape(2, N, D)
        )
    return out


# revision 8
# speedup vs baseline: 1.0684x; 1.0093x over previous
"""Cross-channel multi-head attention on 8 Trainium2 NeuronCores.

Sharding: data-parallel over the batch axis. bs2=16 sequences form bs=8
(batch, 2-channel) pairs; each core handles one pair fully locally
(cross-channel attention couples only the two channels of the same batch
element), so no collectives are needed.

Per core (T=2048 tokens = 2 channels x 1024 patches, D=1024, H=8 heads,
dk=128; heads 0..5 attend to the other channel's K/V, heads 6..7 to the
same channel):
  1. Phase A: V = x @ Wv + bv in natural [T, D] layout, stored per head
     with an extra ones column (softmax denominator trick).
  2. Phase B: per head h the work is round-robined at stretch level --
     scores(h,0) / attnv(h-1,1) / proj(h+1,Q) / scores(h,1) / attnv(h,0)
     / proj(h+1,K) -- so the Act engine's exp stream (the slowest
     per-unit engine) always has 2x PE slack and never blocks a matmul.
     Scores: S^T[m,n] = Kt-slice^T x Qt-slice; P^T = exp(S^T/sqrt(dk));
     attnv: Z-chunks = V-chunks contracted with P^T over m; denominators
     from the ones column; the drain chain (reciprocal/normalize/PE
     transpose into Zt) is software-pipelined two groups behind the
     matmuls.
  3. Phase C: out = Zt^T-slices @ Wo + bo, written bf16 on the Act HWDGE
     ring (host upcasts); Wo chunks 0-3 are preloaded at startup, 4-7
     stream in at C start ahead of the k-outer loop.

All matmuls bf16 with f32 PSUM accumulation. The host pre-arranges x and
the weights so every DMA is >=4KB-per-partition contiguous; all input
DMAs ride one HWDGE queue in consumption order (x/Wv first, then Wq/Wk,
then Wo-low), with biases and output on the Act HWDGE ring.
"""

import sys

if "/opt/trn_rl_repo" not in sys.path:
    sys.path.insert(0, "/opt/trn_rl_repo")

import numpy as np
import ml_dtypes

import concourse.bass as bass
import concourse.tile as tile
from concourse import mybir
from concourse.bass_utils import run_bass_kernel_spmd
from concourse.masks import make_identity

# Walrus in this container rejects >1 wait condition on TPB_CTRL ops
# (Tile's kernel-tail drain carries one per active proc). Split them.
import os

_here = os.path.dirname(os.path.abspath(__file__))
if _here not in sys.path:
    sys.path.insert(0, _here)
try:
    import bir_legalize
except ImportError:  # graded in a bare dir: fall back to inline copy
    bir_legalize = None

N = 1024  # patches per channel
D = 1024
H = 8
DK = 128
N_CROSS = 6
T = 2 * N  # tokens per core (2 channels of one batch element)
P = 128
KO = D // P  # 8 outer chunks of the 1024-wide dims
TC = T // P  # 16 token chunks
NQ = 4  # quarters of the token axis (512 tokens each)
BF = mybir.dt.bfloat16
F32 = mybir.dt.float32
SCALE = 1.0 / float(np.sqrt(DK))

_CACHE = {}


def _legalize_install():
    if bir_legalize is not None:
        bir_legalize.install()
        return
    # Inline fallback (kernel.py must be self-contained when graded).
    import json
    import concourse.bass2jax as bass2jax
    from concourse.bass_utils import compile_bir_kernel as _orig

    if getattr(bass2jax.compile_bir_kernel, "_legalized", False):
        return

    OPCODE_MAX = {}
    SKIP = set()

    def _legalize(bir_json):
        d = json.loads(bir_json)
        changed = False
        for fn in d.get("functions", []):
            for bb in fn.get("blocks") or fn.get("basicblocks") or []:
                out = []
                for inst in bb.get("instructions", []):
                    sync = inst.get("sync_info") or {}
                    waits = sync.get("on_wait") or []
                    cap = OPCODE_MAX.get(inst.get("opcode"), 1)
                    if len(waits) > cap and inst.get("opcode") not in SKIP:
                        extra, keep = waits[:-cap], waits[-cap:]
                        for i, w in enumerate(extra):
                            out.append(
                                {
                                    "debug": inst.get("debug", 0),
                                    "engine": inst["engine"],
                                    "ins": [],
                                    "outs": [],
                                    "is_reset_sema": False,
                                    "name": f"{inst['name']}-sw{i}",
                                    "opcode": "Drain",
                                    "sync_info": {"on_update": [], "on_wait": [w]},
                                }
                            )
                        sync["on_wait"] = keep
                        inst["sync_info"] = sync
                        changed = True
                    out.append(inst)
                bb["instructions"] = out
        return json.dumps(d).encode() if changed else bir_json

    def compile_bir_kernel(bir_json, tmpdir, neff_name="file.neff"):
        return _orig(_legalize(bir_json), tmpdir, neff_name)

    compile_bir_kernel._legalized = True
    bass2jax.compile_bir_kernel = compile_bir_kernel


def _build():
    nc = bass.Bass()

    xt_d = nc.dram_tensor("xt", [P, NQ, KO, 512], BF, kind="ExternalInput").ap()
    wq_d = nc.dram_tensor("wq", [P, KO, D], BF, kind="ExternalInput").ap()
    wk_d = nc.dram_tensor("wk", [P, KO, D], BF, kind="ExternalInput").ap()
    wv_d = nc.dram_tensor("wv", [P, KO, D], BF, kind="ExternalInput").ap()
    wo_d = nc.dram_tensor("wo", [P, KO, D], BF, kind="ExternalInput").ap()
    bqk_d = nc.dram_tensor("bqk", [P, 2 * KO], F32, kind="ExternalInput").ap()
    bvr_d = nc.dram_tensor("bvr", [P, D], F32, kind="ExternalInput").ap()
    bor_d = nc.dram_tensor("bor", [P, D], F32, kind="ExternalInput").ap()
    out_d = nc.dram_tensor("out", [T, D], BF, kind="ExternalOutput").ap()

    with tile.TileContext(nc) as tc:
        with (
            tc.tile_pool(name="consts", bufs=1) as consts,
            tc.tile_pool(name="big", bufs=1) as big,
        ):
            ident = consts.tile([P, P], BF)
            make_identity(nc, ident)
            bqk_sb = consts.tile([P, 2 * KO], F32)
            warm_in = consts.tile([P, P], BF)
            nc.vector.memset(warm_in[:], 0.0)
            warm_rhs = consts.tile([P, 512], BF)
            nc.vector.memset(warm_rhs[:], 0.0)

            Vg = big.tile([P, TC, H, DK + 1], BF)  # natural V + ones col
            nc.vector.memset(Vg[:, :, :, DK : DK + 1], 1.0)
            Zt = big.tile([P, KO, T], BF)  # attention out, [dout, T]
            Wo_lo = big.tile([P, 4, D], BF)  # Wo chunks 0-3, preloaded

            with (
                tc.tile_pool(name="xt_w", bufs=1) as xt_w,
                tc.tile_pool(name="qk2", bufs=2) as qk2,
                tc.tile_pool(name="qk1", bufs=1) as qk1,
                tc.tile_pool(name="pt_pool", bufs=2) as pt_pool,
                tc.tile_pool(name="att_sm", bufs=4) as att_sm,
                tc.tile_pool(name="ps1", bufs=2, space="PSUM") as ps1,
                tc.tile_pool(name="ps_s", bufs=2, space="PSUM") as ps_s,
                tc.tile_pool(name="ps_z", bufs=2, space="PSUM") as ps_z,
                tc.tile_pool(name="ps_zt", bufs=2, space="PSUM") as ps_zt,
            ):
                Xt = xt_w.tile([P, NQ, KO, 512], BF)
                Wq = xt_w.tile([P, KO, D], BF)
                Wk = xt_w.tile([P, KO, D], BF)

                # ---- phase A: V projection ----
                wv_ctx = tc.tile_pool(name="wv_pool", bufs=1)
                wv_pool = wv_ctx.__enter__()
                Wv = wv_pool.tile([P, KO, D], BF)
                bv_r = wv_pool.tile([P, D], F32)

                # Biases on the Act HWDGE ring (parallel to the main
                # input stream on the SP ring).
                nc.scalar.dma_start(bv_r[:], bvr_d)
                nc.scalar.dma_start(bqk_sb[:], bqk_d)

                # Main input stream, in consumption order, >=8KB rows.
                nc.sync.dma_start(Wv[:, 0:4, :], wv_d[:, 0:4, :])
                nc.sync.dma_start(Xt[:, 0, 0:4, :], xt_d[:, 0, 0:4, :])
                nc.sync.dma_start(Wv[:, 4:8, :], wv_d[:, 4:8, :])
                nc.sync.dma_start(Xt[:, 0, 4:8, :], xt_d[:, 0, 4:8, :])
                for q in range(1, NQ):
                    nc.sync.dma_start(Xt[:, q, :, :], xt_d[:, q, :, :])
                for j in range(0, KO, 2):
                    nc.sync.dma_start(Wq[:, j : j + 2, :], wq_d[:, j : j + 2, :])
                    nc.sync.dma_start(Wk[:, j : j + 2, :], wk_d[:, j : j + 2, :])
                nc.sync.dma_start(Wo_lo[:], wo_d[:, 0:4, :])

                # Warm the PE HAM clock gate with throwaway accumulating
                # matmul groups so the first real matmuls run at full
                # clock while the first DMA chunks land.
                for g in range(2):
                    wps = ps_s.tile([P, 512], F32, tag="s")
                    for k in range(12):
                        nc.tensor.matmul(
                            wps[:],
                            warm_in[:],
                            warm_rhs[:],
                            start=(k == 0),
                            stop=(k == 11),
                        )

                # 4 bases x 8 concurrent PSUM groups (borrowing every
                # pool's banks), k-outer so each (x, Wv) chunk pair is
                # consumed across the whole base as soon as it lands.
                groups = [(tci, dh) for tci in range(TC) for dh in range(2)]
                gpools = [ps1, ps1, ps_s, ps_s, ps_z, ps_z, ps_zt, ps_zt]
                gtags = ["ps1", "ps1", "s", "s", "z", "z", "zt", "zt"]
                for base in range(0, len(groups), 8):
                    tiles = [
                        gpools[g].tile(
                            [P, 512], F32, tag=gtags[g], name=f"vps_{base}_{g}"
                        )
                        for g in range(8)
                    ]
                    for k in range(KO):
                        for g in range(8):
                            tci, dh = groups[base + g]
                            nc.tensor.matmul(
                                tiles[g][:],
                                Xt[:, tci // 4, k, (tci % 4) * P : (tci % 4 + 1) * P],
                                Wv[:, k, dh * 512 : (dh + 1) * 512],
                                start=(k == 0),
                                stop=(k == KO - 1),
                            )
                    for g in range(8):
                        tci, dh = groups[base + g]
                        nc.vector.tensor_tensor(
                            Vg[:, tci, 4 * dh : 4 * dh + 4, :DK],
                            tiles[g].rearrange("p (h d) -> p h d", d=DK),
                            bv_r[:, dh * 512 : (dh + 1) * 512].rearrange(
                                "p (h d) -> p h d", d=DK
                            ),
                            mybir.AluOpType.add,
                        )

                wv_ctx.__exit__(None, None, None)

                # ---- phase B ----
                def proj_head(h, w_sb, bias_col):
                    dst = (qk2 if w_sb is Wq else qk1).tile(
                        [P, T], BF, tag="qth" if w_sb is Wq else "kth"
                    )
                    for tt in range(NQ):
                        ps = ps1.tile([P, 512], F32, tag="ps1")
                        for k in range(KO):
                            nc.tensor.matmul(
                                ps[:],
                                w_sb[:, k, h * P : (h + 1) * P],
                                Xt[:, tt, k, :],
                                start=(k == 0),
                                stop=(k == KO - 1),
                            )
                        nc.vector.tensor_tensor(
                            dst[:, tt * 512 : (tt + 1) * 512],
                            ps[:],
                            bqk_sb[:, bias_col : bias_col + 1].to_broadcast((P, 512)),
                            mybir.AluOpType.add,
                        )
                    return dst

                def scores_unit(h, ch, Qth, Kth):
                    chp = (1 - ch) if h < N_CROSS else ch  # kv channel
                    q0 = ch * N
                    m0 = chp * N
                    PT = pt_pool.tile([P, KO, N], BF, tag="pt")
                    for mi in range(KO):
                        for nh in range(2):
                            ps = ps_s.tile([P, 512], F32, tag="s")
                            nc.tensor.matmul(
                                ps[:],
                                Kth[:, m0 + mi * P : m0 + (mi + 1) * P],
                                Qth[:, q0 + nh * 512 : q0 + (nh + 1) * 512],
                                start=True,
                                stop=True,
                            )
                            nc.scalar.activation(
                                PT[:, mi, nh * 512 : (nh + 1) * 512],
                                ps[:],
                                mybir.ActivationFunctionType.Exp,
                                scale=SCALE,
                            )
                    return (PT, h, ch, chp, q0)

                def attnv_unit(state):
                    PT, h, ch, chp, q0 = state
                    pend = {}  # ni -> psz tile
                    pairs = {}  # pj -> [P, 2, P] transpose-pair psum tile

                    def drain(ni):
                        psz = pend.pop(ni)
                        r = att_sm.tile([P, 1], F32, tag="r")
                        nc.vector.reciprocal(r[:], psz[:, DK : DK + 1])
                        zn = att_sm.tile([P, DK], BF, tag="zn")
                        nc.vector.tensor_tensor(
                            zn[:],
                            psz[:, :DK],
                            r[:, 0:1].to_broadcast((P, DK)),
                            mybir.AluOpType.mult,
                        )
                        pj = ni // 2
                        if ni % 2 == 0:
                            pairs[pj] = ps_zt.tile([P, 2, P], BF, tag="zt", name=f"pzt_{h}_{ch}_{pj}")
                        nc.tensor.transpose(pairs[pj][:, ni % 2, :], zn[:], ident[:])

                    def store(pj):
                        nc.vector.tensor_copy(
                            Zt[:, h, q0 + pj * 2 * P : q0 + (pj + 1) * 2 * P],
                            pairs.pop(pj).rearrange("p a b -> p (a b)"),
                        )

                    for ni in range(KO):
                        psz = ps_z.tile([P, DK + 1], F32, tag="z")
                        for mi in range(KO):
                            nc.tensor.matmul(
                                psz[:],
                                PT[:, mi, ni * P : (ni + 1) * P],
                                Vg[:, chp * KO + mi, h, :],
                                start=(mi == 0),
                                stop=(mi == KO - 1),
                            )
                        pend[ni] = psz
                        if ni >= 1:
                            drain(ni - 1)
                            if ni % 2 == 0 and ni >= 2:
                                store(ni // 2 - 1)
                    drain(KO - 1)
                    store(KO // 2 - 1)

                # Stretch-level round-robin: the Act engine's exps for
                # scores(h,ch) drain during the two following Act-free
                # stretches (attnv + proj), so matmuls never wait.
                Qth = proj_head(0, Wq, 0)
                Kth = proj_head(0, Wk, KO + 0)
                prev1 = None  # state (h-1, 1)
                for h in range(H):
                    cur0 = scores_unit(h, 0, Qth, Kth)
                    if prev1 is not None:
                        attnv_unit(prev1)
                    Qn = proj_head(h + 1, Wq, h + 1) if h + 1 < H else None
                    cur1 = scores_unit(h, 1, Qth, Kth)
                    attnv_unit(cur0)
                    Kn = proj_head(h + 1, Wk, KO + h + 1) if h + 1 < H else None
                    Qth, Kth = Qn, Kn
                    prev1 = cur1
                attnv_unit(prev1)

            # ---- phase C: output projection ----
            with (
                tc.tile_pool(name="wo_pool", bufs=1) as wo_pool,
                tc.tile_pool(name="y_pool", bufs=6) as y_pool,
                tc.tile_pool(name="ps_y", bufs=8, space="PSUM") as ps_y,
            ):
                Wo_hi = wo_pool.tile([P, 4, D], BF)
                bo_r = wo_pool.tile([P, D], F32)
                nc.scalar.dma_start(bo_r[:], bor_d)
                nc.sync.dma_start(Wo_hi[:, 0:2, :], wo_d[:, 4:6, :])
                nc.sync.dma_start(Wo_hi[:, 2:4, :], wo_d[:, 6:8, :])

                groups = [(tci, dh) for tci in range(TC) for dh in range(2)]
                ytiles = {}

                def y_drain(g, ps):
                    tci, dh = groups[g]
                    if dh == 0:
                        ytiles[tci] = y_pool.tile([P, D], BF, tag="yt", name=f"yt_{tci}")
                    nc.vector.tensor_tensor(
                        ytiles[tci][:, dh * 512 : (dh + 1) * 512],
                        ps[:],
                        bo_r[:, dh * 512 : (dh + 1) * 512],
                        mybir.AluOpType.add,
                    )
                    if dh == 1:
                        nc.scalar.dma_start(
                            out_d[tci * P : (tci + 1) * P, :], ytiles.pop(tci)[:]
                        )

                # base 0 goes k-outer across 8 concurrent groups so the
                # Wo_hi chunks can stream into the contraction; the rest
                # run k-inner so the output drains spread out and the
                # tail after the last matmul is one tile, not a base.
                tiles = [
                    ps_y.tile([P, 512], F32, tag="y", name=f"yps0_{g}")
                    for g in range(8)
                ]
                for k in range(KO):
                    w_sb = Wo_lo if k < 4 else Wo_hi
                    for g in range(8):
                        tci, dh = groups[g]
                        nc.tensor.matmul(
                            tiles[g][:],
                            Zt[:, k, tci * P : (tci + 1) * P],
                            w_sb[:, k % 4, dh * 512 : (dh + 1) * 512],
                            start=(k == 0),
                            stop=(k == KO - 1),
                        )
                for g in range(8):
                    y_drain(g, tiles[g])
                for g in range(8, len(groups)):
                    tci, dh = groups[g]
                    ps = ps_y.tile([P, 512], F32, tag="y", name=f"yps_{g}")
                    for k in range(KO):
                        w_sb = Wo_lo if k < 4 else Wo_hi
                        nc.tensor.matmul(
                            ps[:],
                            Zt[:, k, tci * P : (tci + 1) * P],
                            w_sb[:, k % 4, dh * 512 : (dh + 1) * 512],
                            start=(k == 0),
                            stop=(k == KO - 1),
                        )
                    y_drain(g, ps)
    return nc


def _get_program():
    if "nc" not in _CACHE:
        _legalize_install()
        _CACHE["nc"] = _build()
    return _CACHE["nc"]


def make_in_maps(inputs):
    x = np.asarray(inputs["x"], dtype=np.float32)
    bs2 = x.shape[0]
    n_cores = bs2 // 2
    bf = ml_dtypes.bfloat16

    weights = {}
    for name in ("Wq", "Wk", "Wv", "Wo"):
        w = np.asarray(inputs[name], dtype=np.float32)
        weights[name] = np.ascontiguousarray(
            w.reshape(KO, P, D).transpose(1, 0, 2)
        ).astype(bf)
    b = {k: np.asarray(inputs[k], dtype=np.float32) for k in ("bq", "bk", "bv", "bo")}
    bqk = np.ascontiguousarray(
        np.concatenate([b["bq"].reshape(KO, P).T, b["bk"].reshape(KO, P).T], axis=1)
    )
    bvr = np.ascontiguousarray(np.broadcast_to(b["bv"], (P, D)))
    bor = np.ascontiguousarray(np.broadcast_to(b["bo"], (P, D)))

    in_maps = []
    for c in range(n_cores):
        xT = x[2 * c : 2 * c + 2].reshape(T, D).T  # [D, T]
        xt = np.ascontiguousarray(
            xT.reshape(KO, P, NQ, 512).transpose(1, 2, 0, 3)
        ).astype(bf)
        in_maps.append(
            {
                "xt": xt,
                "wq": weights["Wq"],
                "wk": weights["Wk"],
                "wv": weights["Wv"],
                "wo": weights["Wo"],
                "bqk": bqk,
                "bvr": bvr,
                "bor": bor,
            }
        )
    return in_maps


def kernel(**inputs):
    bs2 = np.asarray(inputs["x"]).shape[0]
    n_cores = bs2 // 2
    in_maps = make_in_maps(inputs)
    nc = _get_program()
    res = run_bass_kernel_spmd(nc, in_maps, core_ids=list(range(n_cores)))
    out = np.empty((bs2, N, D), dtype=np.float32)
    for c in range(n_cores):
        out[2 * c : 2 * c + 2] = (
            res.results[c]["out"].astype(np.float32).reshape(2, N, D)
        )
    return out


# revision 9
# speedup vs baseline: 1.0714x; 1.0028x over previous
"""Cross-channel multi-head attention on 8 Trainium2 NeuronCores.

Sharding: data-parallel over the batch axis. bs2=16 sequences form bs=8
(batch, 2-channel) pairs; each core handles one pair fully locally
(cross-channel attention couples only the two channels of the same batch
element), so no collectives are needed.

Per core (T=2048 tokens = 2 channels x 1024 patches, D=1024, H=8 heads,
dk=128; heads 0..5 attend to the other channel's K/V, heads 6..7 to the
same channel):
  1. Phase A: V = x @ Wv + bv in natural [T, D] layout, stored per head
     with an extra ones column (softmax denominator trick).
  2. Phase B: per head h the work is round-robined at stretch level --
     scores(h,0) / attnv(h-1,1) / proj(h+1,Q) / scores(h,1) / attnv(h,0)
     / proj(h+1,K) -- so the Act engine's exp stream (the slowest
     per-unit engine) always has 2x PE slack and never blocks a matmul.
     Scores: S^T[m,n] = Kt-slice^T x Qt-slice; P^T = exp(S^T/sqrt(dk));
     attnv: Z-chunks = V-chunks contracted with P^T over m; denominators
     from the ones column; the drain chain (reciprocal/normalize/PE
     transpose into Zt) is software-pipelined two groups behind the
     matmuls.
  3. Phase C: out = Zt^T-slices @ Wo + bo, written bf16 on the Act HWDGE
     ring (host upcasts); Wo chunks 0-3 are preloaded at startup, 4-7
     stream in at C start ahead of the k-outer loop.

All matmuls bf16 with f32 PSUM accumulation. The host pre-arranges x and
the weights so every DMA is >=4KB-per-partition contiguous; all input
DMAs ride one HWDGE queue in consumption order (x/Wv first, then Wq/Wk,
then Wo-low), with biases and output on the Act HWDGE ring.
"""

import sys

if "/opt/trn_rl_repo" not in sys.path:
    sys.path.insert(0, "/opt/trn_rl_repo")

import numpy as np
import ml_dtypes

import concourse.bass as bass
import concourse.tile as tile
from concourse import mybir
from concourse.bass_utils import run_bass_kernel_spmd
from concourse.masks import make_identity

# Walrus in this container rejects >1 wait condition on TPB_CTRL ops
# (Tile's kernel-tail drain carries one per active proc). Split them.
import os

_here = os.path.dirname(os.path.abspath(__file__))
if _here not in sys.path:
    sys.path.insert(0, _here)
try:
    import bir_legalize
except ImportError:  # graded in a bare dir: fall back to inline copy
    bir_legalize = None

N = 1024  # patches per channel
D = 1024
H = 8
DK = 128
N_CROSS = 6
T = 2 * N  # tokens per core (2 channels of one batch element)
P = 128
KO = D // P  # 8 outer chunks of the 1024-wide dims
TC = T // P  # 16 token chunks
NQ = 4  # quarters of the token axis (512 tokens each)
BF = mybir.dt.bfloat16
F32 = mybir.dt.float32
SCALE = 1.0 / float(np.sqrt(DK))

_CACHE = {}


def _legalize_install():
    if bir_legalize is not None:
        bir_legalize.install()
        return
    # Inline fallback (kernel.py must be self-contained when graded).
    import json
    import concourse.bass2jax as bass2jax
    from concourse.bass_utils import compile_bir_kernel as _orig

    if getattr(bass2jax.compile_bir_kernel, "_legalized", False):
        return

    OPCODE_MAX = {}
    SKIP = set()

    def _legalize(bir_json):
        d = json.loads(bir_json)
        changed = False
        for fn in d.get("functions", []):
            for bb in fn.get("blocks") or fn.get("basicblocks") or []:
                out = []
                for inst in bb.get("instructions", []):
                    sync = inst.get("sync_info") or {}
                    waits = sync.get("on_wait") or []
                    cap = OPCODE_MAX.get(inst.get("opcode"), 1)
                    if len(waits) > cap and inst.get("opcode") not in SKIP:
                        extra, keep = waits[:-cap], waits[-cap:]
                        for i, w in enumerate(extra):
                            out.append(
                                {
                                    "debug": inst.get("debug", 0),
                                    "engine": inst["engine"],
                                    "ins": [],
                                    "outs": [],
                                    "is_reset_sema": False,
                                    "name": f"{inst['name']}-sw{i}",
                                    "opcode": "Drain",
                                    "sync_info": {"on_update": [], "on_wait": [w]},
                                }
                            )
                        sync["on_wait"] = keep
                        inst["sync_info"] = sync
                        changed = True
                    out.append(inst)
                bb["instructions"] = out
        return json.dumps(d).encode() if changed else bir_json

    def compile_bir_kernel(bir_json, tmpdir, neff_name="file.neff"):
        return _orig(_legalize(bir_json), tmpdir, neff_name)

    compile_bir_kernel._legalized = True
    bass2jax.compile_bir_kernel = compile_bir_kernel


def _build():
    nc = bass.Bass()

    xt_d = nc.dram_tensor("xt", [P, NQ, KO, 512], BF, kind="ExternalInput").ap()
    wq_d = nc.dram_tensor("wq", [P, KO, D], BF, kind="ExternalInput").ap()
    wk_d = nc.dram_tensor("wk", [P, KO, D], BF, kind="ExternalInput").ap()
    wv_d = nc.dram_tensor("wv", [P, KO, D], BF, kind="ExternalInput").ap()
    wo_d = nc.dram_tensor("wo", [P, KO, D], BF, kind="ExternalInput").ap()
    bqk_d = nc.dram_tensor("bqk", [P, 2 * KO], F32, kind="ExternalInput").ap()
    bvr_d = nc.dram_tensor("bvr", [P, D], F32, kind="ExternalInput").ap()
    bor_d = nc.dram_tensor("bor", [P, D], F32, kind="ExternalInput").ap()
    out_d = nc.dram_tensor("out", [T, D], BF, kind="ExternalOutput").ap()

    with tile.TileContext(nc) as tc:
        with (
            tc.tile_pool(name="consts", bufs=1) as consts,
            tc.tile_pool(name="big", bufs=1) as big,
        ):
            ident = consts.tile([P, P], BF)
            make_identity(nc, ident)
            bqk_sb = consts.tile([P, 2 * KO], F32)
            warm_in = consts.tile([P, P], BF)
            nc.vector.memset(warm_in[:], 0.0)
            warm_rhs = consts.tile([P, 512], BF)
            nc.vector.memset(warm_rhs[:], 0.0)

            Vg = big.tile([P, TC, H, DK + 1], BF)  # natural V + ones col
            nc.vector.memset(Vg[:, :, :, DK : DK + 1], 1.0)
            Zt = big.tile([P, KO, T], BF)  # attention out, [dout, T]
            Wo_lo = big.tile([P, 4, D], BF)  # Wo chunks 0-3, preloaded

            with (
                tc.tile_pool(name="xt_w", bufs=1) as xt_w,
                tc.tile_pool(name="qk2", bufs=2) as qk2,
                tc.tile_pool(name="qk1", bufs=1) as qk1,
                tc.tile_pool(name="pt_pool", bufs=2) as pt_pool,
                tc.tile_pool(name="att_sm", bufs=4) as att_sm,
                tc.tile_pool(name="ps1", bufs=2, space="PSUM") as ps1,
                tc.tile_pool(name="ps_s", bufs=3, space="PSUM") as ps_s,
                tc.tile_pool(name="ps_z", bufs=2, space="PSUM") as ps_z,
                tc.tile_pool(name="ps_zt", bufs=1, space="PSUM") as ps_zt,
            ):
                Xt = xt_w.tile([P, NQ, KO, 512], BF)
                Wq = xt_w.tile([P, KO, D], BF)
                Wk = xt_w.tile([P, KO, D], BF)

                # ---- phase A: V projection ----
                wv_ctx = tc.tile_pool(name="wv_pool", bufs=1)
                wv_pool = wv_ctx.__enter__()
                Wv = wv_pool.tile([P, KO, D], BF)
                bv_r = wv_pool.tile([P, D], F32)

                # Biases on the Act HWDGE ring (parallel to the main
                # input stream on the SP ring).
                nc.scalar.dma_start(bv_r[:], bvr_d)
                nc.scalar.dma_start(bqk_sb[:], bqk_d)

                # Main input stream, in consumption order, >=8KB rows.
                nc.sync.dma_start(Wv[:, 0:4, :], wv_d[:, 0:4, :])
                nc.sync.dma_start(Xt[:, 0, 0:4, :], xt_d[:, 0, 0:4, :])
                nc.sync.dma_start(Wv[:, 4:8, :], wv_d[:, 4:8, :])
                nc.sync.dma_start(Xt[:, 0, 4:8, :], xt_d[:, 0, 4:8, :])
                for q in range(1, NQ):
                    nc.sync.dma_start(Xt[:, q, :, :], xt_d[:, q, :, :])
                for j in range(0, KO, 2):
                    nc.sync.dma_start(Wq[:, j : j + 2, :], wq_d[:, j : j + 2, :])
                    nc.sync.dma_start(Wk[:, j : j + 2, :], wk_d[:, j : j + 2, :])
                nc.sync.dma_start(Wo_lo[:], wo_d[:, 0:4, :])

                # Warm the PE HAM clock gate with throwaway accumulating
                # matmul groups so the first real matmuls run at full
                # clock while the first DMA chunks land.
                for g in range(1):
                    wps = ps_s.tile([P, 512], F32, tag="s")
                    for k in range(12):
                        nc.tensor.matmul(
                            wps[:],
                            warm_in[:],
                            warm_rhs[:],
                            start=(k == 0),
                            stop=(k == 11),
                        )

                # 4 bases x 8 concurrent PSUM groups (borrowing every
                # pool's banks), k-outer so each (x, Wv) chunk pair is
                # consumed across the whole base as soon as it lands.
                groups = [(tci, dh) for tci in range(TC) for dh in range(2)]
                gpools = [ps1, ps1, ps_s, ps_s, ps_s, ps_z, ps_z, ps_zt]
                gtags = ["ps1", "ps1", "s", "s", "s", "z", "z", "zt"]
                for base in range(0, len(groups), 8):
                    tiles = [
                        gpools[g].tile(
                            [P, 512], F32, tag=gtags[g], name=f"vps_{base}_{g}"
                        )
                        for g in range(8)
                    ]
                    for k in range(KO):
                        for g in range(8):
                            tci, dh = groups[base + g]
                            nc.tensor.matmul(
                                tiles[g][:],
                                Xt[:, tci // 4, k, (tci % 4) * P : (tci % 4 + 1) * P],
                                Wv[:, k, dh * 512 : (dh + 1) * 512],
                                start=(k == 0),
                                stop=(k == KO - 1),
                            )
                    for g in range(8):
                        tci, dh = groups[base + g]
                        nc.vector.tensor_tensor(
                            Vg[:, tci, 4 * dh : 4 * dh + 4, :DK],
                            tiles[g].rearrange("p (h d) -> p h d", d=DK),
                            bv_r[:, dh * 512 : (dh + 1) * 512].rearrange(
                                "p (h d) -> p h d", d=DK
                            ),
                            mybir.AluOpType.add,
                        )

                wv_ctx.__exit__(None, None, None)

                # ---- phase B ----
                def proj_head(h, w_sb, bias_col):
                    dst = (qk2 if w_sb is Wq else qk1).tile(
                        [P, T], BF, tag="qth" if w_sb is Wq else "kth"
                    )
                    for tt in range(NQ):
                        ps = ps1.tile([P, 512], F32, tag="ps1")
                        for k in range(KO):
                            nc.tensor.matmul(
                                ps[:],
                                w_sb[:, k, h * P : (h + 1) * P],
                                Xt[:, tt, k, :],
                                start=(k == 0),
                                stop=(k == KO - 1),
                            )
                        nc.vector.tensor_tensor(
                            dst[:, tt * 512 : (tt + 1) * 512],
                            ps[:],
                            bqk_sb[:, bias_col : bias_col + 1].to_broadcast((P, 512)),
                            mybir.AluOpType.add,
                        )
                    return dst

                def scores_unit(h, ch, Qth, Kth):
                    chp = (1 - ch) if h < N_CROSS else ch  # kv channel
                    q0 = ch * N
                    m0 = chp * N
                    PT = pt_pool.tile([P, KO, N], BF, tag="pt")
                    for mi in range(KO):
                        for nh in range(2):
                            ps = ps_s.tile([P, 512], F32, tag="s")
                            nc.tensor.matmul(
                                ps[:],
                                Kth[:, m0 + mi * P : m0 + (mi + 1) * P],
                                Qth[:, q0 + nh * 512 : q0 + (nh + 1) * 512],
                                start=True,
                                stop=True,
                            )
                            nc.scalar.activation(
                                PT[:, mi, nh * 512 : (nh + 1) * 512],
                                ps[:],
                                mybir.ActivationFunctionType.Exp,
                                scale=SCALE,
                            )
                    return (PT, h, ch, chp, q0)

                def attnv_unit(state):
                    PT, h, ch, chp, q0 = state
                    pend = {}  # ni -> psz tile
                    pairs = {}  # pj -> [P, 2, P] transpose-pair psum tile

                    def drain(ni):
                        psz = pend.pop(ni)
                        r = att_sm.tile([P, 1], F32, tag="r")
                        nc.vector.reciprocal(r[:], psz[:, DK : DK + 1])
                        zn = att_sm.tile([P, DK], BF, tag="zn")
                        nc.vector.tensor_tensor(
                            zn[:],
                            psz[:, :DK],
                            r[:, 0:1].to_broadcast((P, DK)),
                            mybir.AluOpType.mult,
                        )
                        pj = ni // 2
                        if ni % 2 == 0:
                            pairs[pj] = ps_zt.tile([P, 2, P], BF, tag="zt", name=f"pzt_{h}_{ch}_{pj}")
                        nc.tensor.transpose(pairs[pj][:, ni % 2, :], zn[:], ident[:])

                    def store(pj):
                        nc.vector.tensor_copy(
                            Zt[:, h, q0 + pj * 2 * P : q0 + (pj + 1) * 2 * P],
                            pairs.pop(pj).rearrange("p a b -> p (a b)"),
                        )

                    for ni in range(KO):
                        psz = ps_z.tile([P, DK + 1], F32, tag="z")
                        for mi in range(KO):
                            nc.tensor.matmul(
                                psz[:],
                                PT[:, mi, ni * P : (ni + 1) * P],
                                Vg[:, chp * KO + mi, h, :],
                                start=(mi == 0),
                                stop=(mi == KO - 1),
                            )
                        pend[ni] = psz
                        if ni >= 1:
                            drain(ni - 1)
                            if ni % 2 == 0 and ni >= 2:
                                store(ni // 2 - 1)
                    drain(KO - 1)
                    store(KO // 2 - 1)

                # Stretch-level round-robin: the Act engine's exps for
                # scores(h,ch) drain during the two following Act-free
                # stretches (attnv + proj), so matmuls never wait.
                Qth = proj_head(0, Wq, 0)
                Kth = proj_head(0, Wk, KO + 0)
                prev1 = None  # state (h-1, 1)
                for h in range(H):
                    cur0 = scores_unit(h, 0, Qth, Kth)
                    if prev1 is not None:
                        attnv_unit(prev1)
                    Qn = proj_head(h + 1, Wq, h + 1) if h + 1 < H else None
                    cur1 = scores_unit(h, 1, Qth, Kth)
                    attnv_unit(cur0)
                    Kn = proj_head(h + 1, Wk, KO + h + 1) if h + 1 < H else None
                    Qth, Kth = Qn, Kn
                    prev1 = cur1
                attnv_unit(prev1)

            # ---- phase C: output projection ----
            with (
                tc.tile_pool(name="wo_pool", bufs=1) as wo_pool,
                tc.tile_pool(name="y_pool", bufs=6) as y_pool,
                tc.tile_pool(name="ps_y", bufs=8, space="PSUM") as ps_y,
            ):
                Wo_hi = wo_pool.tile([P, 4, D], BF)
                bo_r = wo_pool.tile([P, D], F32)
                nc.scalar.dma_start(bo_r[:], bor_d)
                nc.sync.dma_start(Wo_hi[:, 0:2, :], wo_d[:, 4:6, :])
                nc.sync.dma_start(Wo_hi[:, 2:4, :], wo_d[:, 6:8, :])

                groups = [(tci, dh) for tci in range(TC) for dh in range(2)]
                ytiles = {}

                def y_drain(g, ps):
                    tci, dh = groups[g]
                    if dh == 0:
                        ytiles[tci] = y_pool.tile([P, D], BF, tag="yt", name=f"yt_{tci}")
                    nc.vector.tensor_tensor(
                        ytiles[tci][:, dh * 512 : (dh + 1) * 512],
                        ps[:],
                        bo_r[:, dh * 512 : (dh + 1) * 512],
                        mybir.AluOpType.add,
                    )
                    if dh == 1:
                        y = ytiles.pop(tci)
                        if tci >= TC - 2:
                            # shrink the post-last-matmul drain: write the
                            # final tiles in halves so the second half is
                            # all that trails the last matmul
                            nc.scalar.dma_start(
                                out_d[tci * P : (tci + 1) * P, 0:512], y[:, 0:512]
                            )
                            nc.scalar.dma_start(
                                out_d[tci * P : (tci + 1) * P, 512:D], y[:, 512:D]
                            )
                        else:
                            nc.scalar.dma_start(out_d[tci * P : (tci + 1) * P, :], y[:])

                # base 0 goes k-outer across 8 concurrent groups so the
                # Wo_hi chunks can stream into the contraction; the rest
                # run k-inner so the output drains spread out and the
                # tail after the last matmul is one tile, not a base.
                tiles = [
                    ps_y.tile([P, 512], F32, tag="y", name=f"yps0_{g}")
                    for g in range(8)
                ]
                for k in range(KO):
                    w_sb = Wo_lo if k < 4 else Wo_hi
                    for g in range(8):
                        tci, dh = groups[g]
                        nc.tensor.matmul(
                            tiles[g][:],
                            Zt[:, k, tci * P : (tci + 1) * P],
                            w_sb[:, k % 4, dh * 512 : (dh + 1) * 512],
                            start=(k == 0),
                            stop=(k == KO - 1),
                        )
                for g in range(8):
                    y_drain(g, tiles[g])
                for g in range(8, len(groups)):
                    tci, dh = groups[g]
                    ps = ps_y.tile([P, 512], F32, tag="y", name=f"yps_{g}")
                    for k in range(KO):
                        w_sb = Wo_lo if k < 4 else Wo_hi
                        nc.tensor.matmul(
                            ps[:],
                            Zt[:, k, tci * P : (tci + 1) * P],
                            w_sb[:, k % 4, dh * 512 : (dh + 1) * 512],
                            start=(k == 0),
                            stop=(k == KO - 1),
                        )
                    y_drain(g, ps)
    return nc


def _get_program():
    if "nc" not in _CACHE:
        _legalize_install()
        _CACHE["nc"] = _build()
    return _CACHE["nc"]


def make_in_maps(inputs):
    x = np.asarray(inputs["x"], dtype=np.float32)
    bs2 = x.shape[0]
    n_cores = bs2 // 2
    bf = ml_dtypes.bfloat16

    weights = {}
    for name in ("Wq", "Wk", "Wv", "Wo"):
        w = np.asarray(inputs[name], dtype=np.float32)
        weights[name] = np.ascontiguousarray(
            w.reshape(KO, P, D).transpose(1, 0, 2)
        ).astype(bf)
    b = {k: np.asarray(inputs[k], dtype=np.float32) for k in ("bq", "bk", "bv", "bo")}
    bqk = np.ascontiguousarray(
        np.concatenate([b["bq"].reshape(KO, P).T, b["bk"].reshape(KO, P).T], axis=1)
    )
    bvr = np.ascontiguousarray(np.broadcast_to(b["bv"], (P, D)))
    bor = np.ascontiguousarray(np.broadcast_to(b["bo"], (P, D)))

    in_maps = []
    for c in range(n_cores):
        xT = x[2 * c : 2 * c + 2].reshape(T, D).T  # [D, T]
        xt = np.ascontiguousarray(
            xT.reshape(KO, P, NQ, 512).transpose(1, 2, 0, 3)
        ).astype(bf)
        in_maps.append(
            {
                "xt": xt,
                "wq": weights["Wq"],
                "wk": weights["Wk"],
                "wv": weights["Wv"],
                "wo": weights["Wo"],
                "bqk": bqk,
                "bvr": bvr,
                "bor": bor,
            }
        )
    return in_maps


def kernel(**inputs):
    bs2 = np.asarray(inputs["x"]).shape[0]
    n_cores = bs2 // 2
    in_maps = make_in_maps(inputs)
    nc = _get_program()
    res = run_bass_kernel_spmd(nc, in_maps, core_ids=list(range(n_cores)))
    out = np.empty((bs2, N, D), dtype=np.float32)
    for c in range(n_cores):
        out[2 * c : 2 * c + 2] = (
            res.results[c]["out"].astype(np.float32).reshape(2, N, D)
        )
    return out
